# revision 1
# baseline (speedup 1.0000x reference)
"""Multi-head attention kernel for Trainium2, data-parallel over 8 NeuronCores.

Problem: B=16, N=1024, D=768, H=12 heads (hd=64), fp32 I/O.
  qkv = x @ w_qkv + b_qkv ; attention ; out = attn_out @ w_proj + b_proj

Sharding: batch data-parallel — core c handles batches [2c, 2c+2); weights
replicated. Inside each core, the two batches are processed sequentially.

Layout strategy (all compute in f32r on TensorE — tf32-like, ~1.6e-4 rel):
  - host pre-transposes x to xT [768, T] so the in-feature contraction has
    features on partitions for both operands.
  - Q^T, K^T computed feature-major [768, N]: lhsT = w_qkv cols, rhs = xT.
    A 128-row feature tile holds a PAIR of heads (2x64) -> scores matmuls
    for the two heads run concurrently via tile_position row packing (K=64).
  - V computed token-major [N, 768]: lhsT = xT chunk, rhs = w_qkv v-cols,
    stored bf16 with a ones column appended per head (v_ext [128, 65]).
  - scores^T tiles [128 j, 512 q] per head -> one ACT exp op [128, 1024]
    covers both heads of a pair (softmax scale folded into exp's scale).
  - U^T = sum_j exp * v_ext accumulates in PSUM [65, 512]; row 64 is the
    softmax denominator (ones column) — no separate reduction needed.
  - normalize: reciprocal (DVE) -> partition_broadcast (GpSimd) -> multiply
    (DVE), + b_v per-partition. b_q/b_k added at Q^T/K^T evacuation
    (per-partition in feature-major layout).
  - proj: lhsT = attn^T tile, rhs = w_proj; + b_proj via broadcast add.
    Output lands token-major [T, 768] == final layout.
"""

import contextlib
import ctypes
import os
import sys
import types

import numpy as np

# ---------------------------------------------------------------------------
# NTFF profiling shim: bass_utils's trace path imports
# antenv.axon_hooks.get_axon_ntff_profile_hook, which this container's antenv
# lacks. Register a ctypes-based equivalent so BASS_TRACE=1 works. Harmless
# if tracing is never requested.
# ---------------------------------------------------------------------------


def _install_ntff_shim():
    if "antenv.axon_hooks" in sys.modules:
        return
    so_path = "/opt/axon/libaxon_pjrt.so"
    hook = None
    try:
        lib = ctypes.CDLL(so_path)
        if hasattr(lib, "axon_start_nrt_profile"):
            lib.axon_start_nrt_profile.argtypes = [
                ctypes.POINTER(ctypes.c_int64),
                ctypes.c_size_t,
            ]
            lib.axon_start_nrt_profile.restype = ctypes.c_int64
            lib.axon_stop_nrt_profile.argtypes = [ctypes.c_char_p]
            lib.axon_stop_nrt_profile.restype = ctypes.c_int64

            @contextlib.contextmanager
            def _hook(output_dir, device_ids):
                import jax

                jax.devices()
                if device_ids:
                    ids = (ctypes.c_int64 * len(device_ids))(*device_ids)
                    rc = lib.axon_start_nrt_profile(ids, len(device_ids))
                else:
                    rc = lib.axon_start_nrt_profile(None, 0)
                if rc != 0:
                    raise RuntimeError(f"axon_start_nrt_profile rc={rc}")
                try:
                    yield
                finally:
                    n = lib.axon_stop_nrt_profile(str(output_dir).encode())
                    print(f"ntff profile: {n} file(s) in {output_dir}",
                          file=sys.stderr)

            hook = _hook
    except OSError:
        pass
    mod = types.ModuleType("antenv.axon_hooks")
    mod.get_axon_ntff_profile_hook = lambda: hook
    mod.set_axon_ntff_profile_hook = lambda h: None
    sys.modules["antenv.axon_hooks"] = mod


_install_ntff_shim()

import concourse.bacc as bacc  # noqa: E402
import concourse.mybir as mybir  # noqa: E402
import concourse.tile as tile  # noqa: E402
from concourse.bass_utils import run_bass_kernel_spmd  # noqa: E402

F32 = mybir.dt.float32
F32R = mybir.dt.float32r
BF16 = mybir.dt.bfloat16
AF = mybir.ActivationFunctionType

# Problem constants (per core)
NB = 2        # batches per core
TN = 1024     # tokens per batch
T = NB * TN   # tokens per core
D = 768
H = 12
HD = 64
D3 = 3 * D
KT = D // 128          # 6 contraction tiles
NPAIR = H // 2         # 6 head pairs
NJT = TN // 128        # 8 key tiles per batch
SCALE = HD ** -0.5


def build():
    nc = bacc.Bacc(None)
    xT_d = nc.declare_dram_parameter("xT", [D, T], F32, isOutput=False)
    wqkv_d = nc.declare_dram_parameter("wqkv", [D, D3], F32, isOutput=False)
    wproj_d = nc.declare_dram_parameter("wproj", [D, D], F32, isOutput=False)
    bqk_d = nc.declare_dram_parameter("bqk", [128, 12], F32, isOutput=False)
    bv_d = nc.declare_dram_parameter("bv", [128, 6], F32, isOutput=False)
    bproj_d = nc.declare_dram_parameter("bproj", [1, D], F32, isOutput=False)
    ones_d = nc.declare_dram_parameter("ones", [128, 96], F32, isOutput=False)
    out_d = nc.declare_dram_parameter("out", [T, D], F32, isOutput=True)

    with tile.TileContext(nc) as tc:
        with (
            nc.allow_low_precision(reason="f32r/bf16 attention pipeline"),
            tc.tile_pool(name="const", bufs=1) as cpool,
            tc.tile_pool(name="xu", bufs=2) as xupool,
            tc.tile_pool(name="qk", bufs=1) as qkpool,
            tc.tile_pool(name="vsb", bufs=1) as vpool,
            tc.tile_pool(name="esb", bufs=3) as epool,
            tc.tile_pool(name="rsb", bufs=2) as rpool,
            tc.tile_pool(name="bsb", bufs=2) as bpool,
            tc.tile_pool(name="osb", bufs=2) as opool,
            tc.tile_pool(name="psS", bufs=2, space="PSUM") as psS,
            tc.tile_pool(name="psU", bufs=2, space="PSUM") as psU,
            tc.tile_pool(name="psQ", bufs=2, space="PSUM") as psQ,
        ):
            # ---- constants / weights (resident) ----
            wqkv = cpool.tile([128, KT, D3], F32R, tag="wqkv")
            nc.gpsimd.dma_start(
                wqkv[:], wqkv_d.ap().rearrange("(ko p) n -> p ko n", p=128)
            )
            wproj = cpool.tile([128, KT, D], F32R, tag="wproj")
            nc.gpsimd.dma_start(
                wproj[:], wproj_d.ap().rearrange("(ko p) n -> p ko n", p=128)
            )
            bqk = cpool.tile([128, 12], F32, tag="bqk")
            nc.sync.dma_start(bqk[:], bqk_d.ap())
            bv = cpool.tile([128, 6], F32, tag="bv")
            nc.sync.dma_start(bv[:], bv_d.ap())
            bproj1 = cpool.tile([1, D], F32, tag="bproj1")
            nc.sync.dma_start(bproj1[:], bproj_d.ap())
            bprojb = cpool.tile([128, D], F32, tag="bprojb")
            nc.gpsimd.partition_broadcast(bprojb[:], bproj1[:])

            qT = qkpool.tile([128, NPAIR, TN], F32R, tag="qT")
            kT = qkpool.tile([128, NPAIR, TN], F32R, tag="kT")
            vsb = vpool.tile([128, NJT, H, HD + 1], BF16, tag="v")
            # ones columns of v_ext (col 64 of every (jt, h) slot)
            nc.gpsimd.dma_start(
                vsb[:, :, :, HD : HD + 1],
                ones_d.ap().rearrange("p (a b) -> p a b", a=NJT),
            )

            for b in range(NB):
                tok0 = b * TN
                # ---- stage A: load xT, compute Q^T / K^T / V ----
                xTb = xupool.tile([128, KT, TN], F32R, tag="xu", name=f"xT{b}")
                nc.gpsimd.dma_start(
                    xTb[:],
                    xT_d.ap().rearrange("(ko p) n -> p ko n", p=128)[
                        :, :, tok0 : tok0 + TN
                    ],
                )

                # Q^T and K^T: feature-major. m-tile m holds heads (2m, 2m+1)
                # for Q (m<6) or K (m-6).
                for m in range(12):
                    dst = qT if m < 6 else kT
                    hp = m % 6
                    for ih in range(2):
                        ps = psQ.tile([128, 512], F32, tag="ps")
                        for k in range(KT):
                            nc.tensor.matmul(
                                ps[:],
                                wqkv[:, k, m * 128 : (m + 1) * 128],
                                xTb[:, k, ih * 512 : (ih + 1) * 512],
                                start=(k == 0),
                                stop=(k == KT - 1),
                            )
                        nc.vector.tensor_scalar_add(
                            dst[:, hp, ih * 512 : (ih + 1) * 512],
                            ps[:],
                            bqk[:, m : m + 1],
                        )

                # V token-major, stored bf16 into v_ext slots (no bias here;
                # b_v is added after normalization, where it is per-partition)
                for t in range(NJT):
                    for nh in range(2):
                        ps = psQ.tile([128, 384], F32, tag="ps")
                        for k in range(KT):
                            nc.tensor.matmul(
                                ps[:],
                                xTb[:, k, t * 128 : (t + 1) * 128],
                                wqkv[:, k, 2 * D + nh * 384 : 2 * D + (nh + 1) * 384],
                                start=(k == 0),
                                stop=(k == KT - 1),
                            )
                        # [128, 384] -> heads nh*6..nh*6+5, cols 0:64 of v_ext
                        nc.vector.tensor_copy(
                            vsb[:, t, nh * 6 : (nh + 1) * 6, 0:HD], ps[:]
                        )

                # ---- stage B: attention, per head pair, per i-half ----
                uT = xupool.tile([128, KT, TN], F32R, tag="xu", name=f"uT{b}")
                for hp in range(NPAIR):
                    for ih in range(2):
                        i0 = ih * 512
                        pu = [
                            psU.tile([HD + 1, 512], F32, tag="pu", name=f"pu{h}")
                            for h in range(2)
                        ]
                        for jt in range(NJT):
                            # scores^T for both heads of the pair, row-packed
                            ps = psS.tile([128, 1024], F32, tag="s")
                            for h in range(2):
                                nc.tensor.matmul(
                                    ps[:, h * 512 : (h + 1) * 512],
                                    kT[
                                        h * 64 : (h + 1) * 64,
                                        hp,
                                        jt * 128 : (jt + 1) * 128,
                                    ],
                                    qT[h * 64 : (h + 1) * 64, hp, i0 : i0 + 512],
                                )
                            e = epool.tile([128, 1024], BF16, tag="e")
                            nc.scalar.activation(e[:], ps[:], AF.Exp, scale=SCALE)
                            for h in range(2):
                                nc.tensor.matmul(
                                    pu[h][:],
                                    vsb[:, jt, 2 * hp + h, :],
                                    e[:, h * 512 : (h + 1) * 512],
                                    start=(jt == 0),
                                    stop=(jt == NJT - 1),
                                )
                        # normalize + b_v -> uT (feature-major attn output)
                        for h in range(2):
                            hh = 2 * hp + h
                            rec = rpool.tile([1, 512], F32, tag="rec")
                            nc.vector.reciprocal(rec[:], pu[h][HD : HD + 1, :])
                            rb = bpool.tile([128, 512], F32, tag="rb")
                            nc.gpsimd.partition_broadcast(rb[:], rec[:])
                            usl = uT[
                                h * 64 : (h + 1) * 64, hp, i0 : i0 + 512
                            ]
                            nc.vector.tensor_mul(
                                usl, pu[h][0:HD, :], rb[h * 64 : (h + 1) * 64, :]
                            )
                            nc.vector.tensor_scalar_add(
                                usl, usl, bv[h * 64 : (h + 1) * 64, hp : hp + 1]
                            )

                # ---- stage P: output projection ----
                for t in range(NJT):
                    ot = opool.tile([128, D], F32, tag="o")
                    for nh in range(2):
                        ps = psQ.tile([128, 384], F32, tag="ps")
                        for k in range(KT):
                            nc.tensor.matmul(
                                ps[:],
                                uT[:, k, t * 128 : (t + 1) * 128],
                                wproj[:, k, nh * 384 : (nh + 1) * 384],
                                start=(k == 0),
                                stop=(k == KT - 1),
                            )
                        nc.vector.tensor_add(
                            ot[:, nh * 384 : (nh + 1) * 384],
                            ps[:],
                            bprojb[:, nh * 384 : (nh + 1) * 384],
                        )
                    nc.sync.dma_start(
                        out_d.ap()[tok0 + t * 128 : tok0 + (t + 1) * 128, :], ot[:]
                    )

    nc.compile()
    return nc


_NC_CACHE = None


def _get_nc():
    global _NC_CACHE
    if _NC_CACHE is None:
        _NC_CACHE = build()
    return _NC_CACHE


def _prep_core_inputs(x_c, w_qkv, b_qkv, w_proj, b_proj):
    """Host-side layout prep for one core. x_c: [2, 1024, 768]."""
    xT = np.ascontiguousarray(x_c.reshape(T, D).T).astype(np.float32)
    bqk = np.ascontiguousarray(b_qkv[: 12 * 128].reshape(12, 128).T)
    # bv: [128, 6]; column hp = b_v for heads (2hp, 2hp+1) stacked
    bv = np.ascontiguousarray(
        b_qkv[2 * D :].reshape(6, 128).T
    )  # b_v[128*hp + p] at [p, hp] == heads 2hp (p<64), 2hp+1 (p>=64)
    return {
        "xT": xT,
        "wqkv": np.ascontiguousarray(w_qkv, dtype=np.float32),
        "wproj": np.ascontiguousarray(w_proj, dtype=np.float32),
        "bqk": bqk.astype(np.float32),
        "bv": bv.astype(np.float32),
        "bproj": np.ascontiguousarray(b_proj.reshape(1, D), dtype=np.float32),
        "ones": np.ones((128, 96), dtype=np.float32),
    }


def kernel(x, w_qkv, b_qkv, w_proj, b_proj):
    x = np.asarray(x, dtype=np.float32)
    w_qkv = np.asarray(w_qkv, dtype=np.float32)
    b_qkv = np.asarray(b_qkv, dtype=np.float32)
    w_proj = np.asarray(w_proj, dtype=np.float32)
    b_proj = np.asarray(b_proj, dtype=np.float32)
    B, N, Dd = x.shape
    assert (B, N, Dd) == (16, 1024, 768)

    nc = _get_nc()
    in_maps = [
        _prep_core_inputs(x[2 * c : 2 * c + 2], w_qkv, b_qkv, w_proj, b_proj)
        for c in range(8)
    ]
    res = run_bass_kernel_spmd(nc, in_maps, core_ids=list(range(8)))
    out = np.empty((B, N, Dd), dtype=np.float32)
    for c in range(8):
        out[2 * c : 2 * c + 2] = res.results[c]["out"].reshape(2, N, Dd)
    kernel.last_results = res
    return out


# revision 10
# speedup vs baseline: 1.1033x; 1.1033x over previous
"""Multi-head attention kernel for Trainium2, data-parallel over 8 NeuronCores.

Problem: B=16, N=1024, D=768, H=12 heads (hd=64), fp32 I/O.
  qkv = x @ w_qkv + b_qkv ; attention ; out = attn_out @ w_proj + b_proj

Sharding: batch data-parallel — core c handles batches [2c, 2c+2); weights
replicated. Inside each core, the two batches are processed sequentially.

Layout strategy (all compute in f32r on TensorE — tf32-like, ~1.6e-4 rel):
  - host pre-transposes x to xT [768, T] so the in-feature contraction has
    features on partitions for both operands.
  - Q^T, K^T computed feature-major [768, N]: lhsT = w_qkv cols, rhs = xT.
    A 128-row feature tile holds a PAIR of heads (2x64) -> scores matmuls
    for the two heads run concurrently via tile_position row packing (K=64).
  - V computed token-major [N, 768]: lhsT = xT chunk, rhs = w_qkv v-cols,
    stored bf16 with a ones column appended per head (v_ext [128, 65]).
  - scores^T tiles [128 j, 512 q] per head -> one ACT exp op [128, 1024]
    covers both heads of a pair (softmax scale folded into exp's scale).
  - U^T = sum_j exp * v_ext accumulates in PSUM [65, 512]; row 64 is the
    softmax denominator (ones column) — no separate reduction needed.
  - normalize: reciprocal (DVE) -> partition_broadcast (GpSimd) -> multiply
    (DVE), + b_v per-partition. b_q/b_k added at Q^T/K^T evacuation
    (per-partition in feature-major layout).
  - proj: lhsT = attn^T tile, rhs = w_proj; + b_proj via broadcast add.
    Output lands token-major [T, 768] == final layout.
"""

import contextlib
import ctypes
import os
import sys
import types

import numpy as np

# ---------------------------------------------------------------------------
# NTFF profiling shim: bass_utils's trace path imports
# antenv.axon_hooks.get_axon_ntff_profile_hook, which this container's antenv
# lacks. Register a ctypes-based equivalent so BASS_TRACE=1 works. Harmless
# if tracing is never requested.
# ---------------------------------------------------------------------------


def _install_ntff_shim():
    if "antenv.axon_hooks" in sys.modules:
        return
    so_path = "/opt/axon/libaxon_pjrt.so"
    hook = None
    try:
        lib = ctypes.CDLL(so_path)
        if hasattr(lib, "axon_start_nrt_profile"):
            lib.axon_start_nrt_profile.argtypes = [
                ctypes.POINTER(ctypes.c_int64),
                ctypes.c_size_t,
            ]
            lib.axon_start_nrt_profile.restype = ctypes.c_int64
            lib.axon_stop_nrt_profile.argtypes = [ctypes.c_char_p]
            lib.axon_stop_nrt_profile.restype = ctypes.c_int64

            @contextlib.contextmanager
            def _hook(output_dir, device_ids):
                import jax

                jax.devices()
                if device_ids:
                    ids = (ctypes.c_int64 * len(device_ids))(*device_ids)
                    rc = lib.axon_start_nrt_profile(ids, len(device_ids))
                else:
                    rc = lib.axon_start_nrt_profile(None, 0)
                if rc != 0:
                    raise RuntimeError(f"axon_start_nrt_profile rc={rc}")
                try:
                    yield
                finally:
                    n = lib.axon_stop_nrt_profile(str(output_dir).encode())
                    print(f"ntff profile: {n} file(s) in {output_dir}",
                          file=sys.stderr)

            hook = _hook
    except OSError:
        pass
    mod = types.ModuleType("antenv.axon_hooks")
    mod.get_axon_ntff_profile_hook = lambda: hook
    mod.set_axon_ntff_profile_hook = lambda h: None
    sys.modules["antenv.axon_hooks"] = mod


_install_ntff_shim()

import concourse.bacc as bacc  # noqa: E402
import concourse.mybir as mybir  # noqa: E402
import concourse.tile as tile  # noqa: E402
from concourse.bass_utils import run_bass_kernel_spmd  # noqa: E402

F32 = mybir.dt.float32
F32R = mybir.dt.float32r
BF16 = mybir.dt.bfloat16
AF = mybir.ActivationFunctionType

# Problem constants (per core)
NB = 2        # batches per core
TN = 1024     # tokens per batch
T = NB * TN   # tokens per core
D = 768
H = 12
HD = 64
D3 = 3 * D
KT = D // 128          # 6 contraction tiles
NPAIR = H // 2         # 6 head pairs
NJT = TN // 128        # 8 key tiles per batch
SCALE = HD ** -0.5


def build():
    nc = bacc.Bacc(None)
    xT_d = nc.declare_dram_parameter("xT", [D, T], F32, isOutput=False)
    wqkv_d = nc.declare_dram_parameter("wqkv", [D, D3], F32, isOutput=False)
    wproj_d = nc.declare_dram_parameter("wproj", [D, D], F32, isOutput=False)
    bqk_d = nc.declare_dram_parameter("bqk", [128, 12], F32, isOutput=False)
    bv_d = nc.declare_dram_parameter("bv", [1, D], F32, isOutput=False)
    bproj_d = nc.declare_dram_parameter("bproj", [1, D], F32, isOutput=False)
    ones_d = nc.declare_dram_parameter("ones", [128, 96], F32, isOutput=False)
    out_d = nc.declare_dram_parameter("out", [T, D], F32, isOutput=True)

    with tile.TileContext(nc) as tc:
        with (
            nc.allow_low_precision(reason="f32r/bf16 attention pipeline"),
            tc.tile_pool(name="const", bufs=1) as cpool,
            tc.tile_pool(name="xu", bufs=2) as xupool,
            tc.tile_pool(name="qk", bufs=1) as qkpool,
            tc.tile_pool(name="vsb", bufs=1) as vpool,
            tc.tile_pool(name="esb", bufs=2) as epool,
            tc.tile_pool(name="stg", bufs=2) as spool,
            tc.tile_pool(name="gat", bufs=2) as gpool,
            tc.tile_pool(name="bsb", bufs=1) as bpool,
            tc.tile_pool(name="osb", bufs=2) as opool,
            tc.tile_pool(name="psS", bufs=2, space="PSUM") as psS,
            tc.tile_pool(name="psU", bufs=2, space="PSUM") as psU,
            tc.tile_pool(name="psQ", bufs=2, space="PSUM") as psQ,
        ):
            # ---- constants / weights (resident) ----
            wqkv = cpool.tile([128, KT, D3], F32R, tag="wqkv")
            nc.gpsimd.dma_start(
                wqkv[:], wqkv_d.ap().rearrange("(ko p) n -> p ko n", p=128)
            )
            wproj = cpool.tile([128, KT, D], F32R, tag="wproj")
            nc.gpsimd.dma_start(
                wproj[:], wproj_d.ap().rearrange("(ko p) n -> p ko n", p=128)
            )
            bqk = cpool.tile([128, 12], F32, tag="bqk")
            nc.sync.dma_start(bqk[:], bqk_d.ap())
            bv1 = cpool.tile([1, D], BF16, tag="bv1")
            nc.gpsimd.dma_start(bv1[:], bv_d.ap())
            bvb = cpool.tile([128, D], BF16, tag="bvb")
            nc.gpsimd.partition_broadcast(bvb[:], bv1[:])
            bproj1 = cpool.tile([1, D], BF16, tag="bproj1")
            nc.gpsimd.dma_start(bproj1[:], bproj_d.ap())
            bprojb = cpool.tile([128, D], BF16, tag="bprojb")
            nc.gpsimd.partition_broadcast(bprojb[:], bproj1[:])

            qT = qkpool.tile([128, NPAIR, TN], F32R, tag="qT")
            kT = qkpool.tile([128, NPAIR, TN], F32R, tag="kT")
            vsb = vpool.tile([128, NJT, H, HD + 1], BF16, tag="v")
            # ones columns of v_ext (col 64 of every (jt, h) slot)
            nc.gpsimd.dma_start(
                vsb[:, :, :, HD : HD + 1],
                ones_d.ap().rearrange("p (a b) -> p a b", a=NJT),
            )

            for b in range(NB):
                tok0 = b * TN
                # ---- stage A: load xT, compute Q^T / K^T / V ----
                xTb = xupool.tile([128, KT, TN], F32R, tag="xu", name=f"xT{b}")
                nc.gpsimd.dma_start(
                    xTb[:],
                    xT_d.ap().rearrange("(ko p) n -> p ko n", p=128)[
                        :, :, tok0 : tok0 + TN
                    ],
                )

                # Q^T and K^T: feature-major. m-tile m holds heads (2m, 2m+1)
                # for Q (m<6) or K (m-6).
                for m in range(12):
                    dst = qT if m < 6 else kT
                    hp = m % 6
                    for ih in range(2):
                        ps = psQ.tile([128, 512], F32, tag="ps")
                        for k in range(KT):
                            nc.tensor.matmul(
                                ps[:],
                                wqkv[:, k, m * 128 : (m + 1) * 128],
                                xTb[:, k, ih * 512 : (ih + 1) * 512],
                                start=(k == 0),
                                stop=(k == KT - 1),
                            )
                        nc.vector.tensor_scalar_add(
                            dst[:, hp, ih * 512 : (ih + 1) * 512],
                            ps[:],
                            bqk[:, m : m + 1],
                        )

                # V token-major, stored bf16 into v_ext slots (no bias here;
                # b_v is added after normalization, where it is per-partition)
                for t in range(NJT):
                    for nh in range(2):
                        ps = psQ.tile([128, 384], F32, tag="ps")
                        for k in range(KT):
                            nc.tensor.matmul(
                                ps[:],
                                xTb[:, k, t * 128 : (t + 1) * 128],
                                wqkv[:, k, 2 * D + nh * 384 : 2 * D + (nh + 1) * 384],
                                start=(k == 0),
                                stop=(k == KT - 1),
                            )
                        # [128, 384] -> heads nh*6..nh*6+5, cols 0:64 of
                        # v_ext, + b_v (b_v then flows through exp@v_ext and
                        # the ones-column normalization exactly)
                        nc.vector.tensor_add(
                            vsb[:, t, nh * 6 : (nh + 1) * 6, 0:HD],
                            ps[:],
                            bvb[:, nh * 384 : (nh + 1) * 384],
                        )

                # ---- stage B: attention, per head pair, per i-half ----
                uT = xupool.tile([128, KT, TN], F32R, tag="xu", name=f"uT{b}")
                for hp in range(NPAIR):
                    for ih in range(2):
                        i0 = ih * 512
                        pu = [
                            psU.tile([HD + 1, 512], F32, tag="pu", name=f"pu{h}")
                            for h in range(2)
                        ]
                        for jt in range(NJT):
                            # scores^T for both heads of the pair, row-packed
                            ps = psS.tile([128, 1024], F32, tag="s")
                            for h in range(2):
                                nc.tensor.matmul(
                                    ps[:, h * 512 : (h + 1) * 512],
                                    kT[
                                        h * 64 : (h + 1) * 64,
                                        hp,
                                        jt * 128 : (jt + 1) * 128,
                                    ],
                                    qT[h * 64 : (h + 1) * 64, hp, i0 : i0 + 512],
                                )
                            e = epool.tile([128, 1024], BF16, tag="e")
                            nc.scalar.activation(e[:], ps[:], AF.Exp, scale=SCALE)
                            for h in range(2):
                                nc.tensor.matmul(
                                    pu[h][:],
                                    vsb[:, jt, 2 * hp + h, :],
                                    e[:, h * 512 : (h + 1) * 512],
                                    start=(jt == 0),
                                    stop=(jt == NJT - 1),
                                )
                        # Evacuate U+r fast (releases PSUM so the next B2
                        # group starts immediately — keeps TensorE dense and
                        # HAM warm), then normalize off the critical path.
                        # Both heads' denominator rows go to partitions 0/32
                        # of one gather tile so a single (expensive, ~6cpe)
                        # exact reciprocal covers them.
                        g = gpool.tile([33, 512], F32, tag="g")
                        nc.gpsimd.memset(g[:], 1.0)
                        usts = []
                        for h in range(2):
                            ust = spool.tile([HD, 512], F32, tag="ust")
                            nc.vector.tensor_copy(ust[:], pu[h][0:HD, :])
                            nc.vector.tensor_copy(
                                g[32 * h : 32 * h + 1, :], pu[h][HD : HD + 1, :]
                            )
                            usts.append(ust)
                        rc = gpool.tile([33, 512], F32, tag="g")
                        nc.vector.reciprocal(rc[:], g[:])
                        # partition_broadcast on HW broadcasts the tile's
                        # absolute partition 0 (ignores AP base) — shift h1's
                        # recip row down into the dead gather tile first.
                        nc.vector.tensor_copy(g[0:1, :], rc[32:33, :])
                        for h in range(2):
                            rb = bpool.tile([128, 512], F32, tag="rb")
                            nc.gpsimd.partition_broadcast(
                                rb[:], rc[0:1, :] if h == 0 else g[0:1, :]
                            )
                            usl = uT[
                                h * 64 : (h + 1) * 64, hp, i0 : i0 + 512
                            ]
                            nc.vector.tensor_mul(usl, usts[h][:], rb[0:HD, :])

                # ---- stage P: output projection ----
                for t in range(NJT):
                    for nh in range(2):
                        ps = psQ.tile([128, 384], F32, tag="ps")
                        for k in range(KT):
                            nc.tensor.matmul(
                                ps[:],
                                uT[:, k, t * 128 : (t + 1) * 128],
                                wproj[:, k, nh * 384 : (nh + 1) * 384],
                                start=(k == 0),
                                stop=(k == KT - 1),
                            )
                        ot = opool.tile([128, 384], F32, tag="o")
                        nc.vector.tensor_add(
                            ot[:],
                            ps[:],
                            bprojb[:, nh * 384 : (nh + 1) * 384],
                        )
                        nc.sync.dma_start(
                            out_d.ap()[
                                tok0 + t * 128 : tok0 + (t + 1) * 128,
                                nh * 384 : (nh + 1) * 384,
                            ],
                            ot[:],
                        )

    nc.compile()
    return nc


_NC_CACHE = None


def _get_nc():
    global _NC_CACHE
    if _NC_CACHE is None:
        _NC_CACHE = build()
    return _NC_CACHE


def _prep_core_inputs(x_c, w_qkv, b_qkv, w_proj, b_proj):
    """Host-side layout prep for one core. x_c: [2, 1024, 768]."""
    xT = np.ascontiguousarray(x_c.reshape(T, D).T).astype(np.float32)
    bqk = np.ascontiguousarray(b_qkv[: 12 * 128].reshape(12, 128).T)
    return {
        "xT": xT,
        "wqkv": np.ascontiguousarray(w_qkv, dtype=np.float32),
        "wproj": np.ascontiguousarray(w_proj, dtype=np.float32),
        "bqk": bqk.astype(np.float32),
        "bv": np.ascontiguousarray(b_qkv[2 * D :].reshape(1, D), dtype=np.float32),
        "bproj": np.ascontiguousarray(b_proj.reshape(1, D), dtype=np.float32),
        "ones": np.ones((128, 96), dtype=np.float32),
    }


def kernel(x, w_qkv, b_qkv, w_proj, b_proj):
    x = np.asarray(x, dtype=np.float32)
    w_qkv = np.asarray(w_qkv, dtype=np.float32)
    b_qkv = np.asarray(b_qkv, dtype=np.float32)
    w_proj = np.asarray(w_proj, dtype=np.float32)
    b_proj = np.asarray(b_proj, dtype=np.float32)
    B, N, Dd = x.shape
    assert (B, N, Dd) == (16, 1024, 768)

    nc = _get_nc()
    in_maps = [
        _prep_core_inputs(x[2 * c : 2 * c + 2], w_qkv, b_qkv, w_proj, b_proj)
        for c in range(8)
    ]
    res = run_bass_kernel_spmd(nc, in_maps, core_ids=list(range(8)))
    out = np.empty((B, N, Dd), dtype=np.float32)
    for c in range(8):
        out[2 * c : 2 * c + 2] = res.results[c]["out"].reshape(2, N, Dd)
    kernel.last_results = res
    return out


# revision 12
# speedup vs baseline: 1.1966x; 1.0845x over previous
"""Multi-head attention kernel for Trainium2, data-parallel over 8 NeuronCores.

Problem: B=16, N=1024, D=768, H=12 heads (hd=64), fp32 I/O.
  qkv = x @ w_qkv + b_qkv ; attention ; out = attn_out @ w_proj + b_proj

Sharding: batch data-parallel — core c handles batches [2c, 2c+2); weights
replicated. Inside each core, the two batches are processed sequentially.

Layout strategy (all compute in f32r on TensorE — tf32-like, ~1.6e-4 rel):
  - host pre-transposes x to xT [768, T] so the in-feature contraction has
    features on partitions for both operands.
  - Q^T, K^T computed feature-major [768, N]: lhsT = w_qkv cols, rhs = xT.
    A 128-row feature tile holds a PAIR of heads (2x64) -> scores matmuls
    for the two heads run concurrently via tile_position row packing (K=64).
  - V computed token-major [N, 768]: lhsT = xT chunk, rhs = w_qkv v-cols,
    stored bf16 with a ones column appended per head (v_ext [128, 65]).
  - scores^T tiles [128 j, 512 q] per head -> one ACT exp op [128, 1024]
    covers both heads of a pair (softmax scale folded into exp's scale).
  - U^T = sum_j exp * v_ext accumulates in PSUM [65, 512]; row 64 is the
    softmax denominator (ones column) — no separate reduction needed.
  - normalize: reciprocal (DVE) -> partition_broadcast (GpSimd) -> multiply
    (DVE), + b_v per-partition. b_q/b_k added at Q^T/K^T evacuation
    (per-partition in feature-major layout).
  - proj: lhsT = attn^T tile, rhs = w_proj; + b_proj via broadcast add.
    Output lands token-major [T, 768] == final layout.
"""

import contextlib
import ctypes
import os
import sys
import types

import numpy as np

# ---------------------------------------------------------------------------
# NTFF profiling shim: bass_utils's trace path imports
# antenv.axon_hooks.get_axon_ntff_profile_hook, which this container's antenv
# lacks. Register a ctypes-based equivalent so BASS_TRACE=1 works. Harmless
# if tracing is never requested.
# ---------------------------------------------------------------------------


def _install_ntff_shim():
    if "antenv.axon_hooks" in sys.modules:
        return
    so_path = "/opt/axon/libaxon_pjrt.so"
    hook = None
    try:
        lib = ctypes.CDLL(so_path)
        if hasattr(lib, "axon_start_nrt_profile"):
            lib.axon_start_nrt_profile.argtypes = [
                ctypes.POINTER(ctypes.c_int64),
                ctypes.c_size_t,
            ]
            lib.axon_start_nrt_profile.restype = ctypes.c_int64
            lib.axon_stop_nrt_profile.argtypes = [ctypes.c_char_p]
            lib.axon_stop_nrt_profile.restype = ctypes.c_int64

            @contextlib.contextmanager
            def _hook(output_dir, device_ids):
                import jax

                jax.devices()
                if device_ids:
                    ids = (ctypes.c_int64 * len(device_ids))(*device_ids)
                    rc = lib.axon_start_nrt_profile(ids, len(device_ids))
                else:
                    rc = lib.axon_start_nrt_profile(None, 0)
                if rc != 0:
                    raise RuntimeError(f"axon_start_nrt_profile rc={rc}")
                try:
                    yield
                finally:
                    n = lib.axon_stop_nrt_profile(str(output_dir).encode())
                    print(f"ntff profile: {n} file(s) in {output_dir}",
                          file=sys.stderr)

            hook = _hook
    except OSError:
        pass
    mod = types.ModuleType("antenv.axon_hooks")
    mod.get_axon_ntff_profile_hook = lambda: hook
    mod.set_axon_ntff_profile_hook = lambda h: None
    sys.modules["antenv.axon_hooks"] = mod


_install_ntff_shim()

import concourse.bacc as bacc  # noqa: E402
import concourse.mybir as mybir  # noqa: E402
import concourse.tile as tile  # noqa: E402
from concourse.bass_utils import run_bass_kernel_spmd  # noqa: E402

F32 = mybir.dt.float32
F32R = mybir.dt.float32r
BF16 = mybir.dt.bfloat16
AF = mybir.ActivationFunctionType

# Problem constants (per core)
NB = 2        # batches per core
TN = 1024     # tokens per batch
T = NB * TN   # tokens per core
D = 768
H = 12
HD = 64
D3 = 3 * D
KT = D // 128          # 6 contraction tiles
NPAIR = H // 2         # 6 head pairs
NJT = TN // 128        # 8 key tiles per batch
SCALE = HD ** -0.5


def build():
    nc = bacc.Bacc(None)
    xT_d = nc.declare_dram_parameter("xT", [D, T], F32, isOutput=False)
    wqkv_d = nc.declare_dram_parameter("wqkv", [D, D3], F32, isOutput=False)
    wproj_d = nc.declare_dram_parameter("wproj", [D, D], F32, isOutput=False)
    bqk_d = nc.declare_dram_parameter("bqk", [128, 12], F32, isOutput=False)
    bv_d = nc.declare_dram_parameter("bv", [1, D], F32, isOutput=False)
    bproj_d = nc.declare_dram_parameter("bproj", [1, D], F32, isOutput=False)
    ones_d = nc.declare_dram_parameter("ones", [128, 96], F32, isOutput=False)
    out_d = nc.declare_dram_parameter("out", [T, D], F32, isOutput=True)

    with tile.TileContext(nc) as tc:
        with (
            nc.allow_low_precision(reason="f32r/bf16 attention pipeline"),
            tc.tile_pool(name="const", bufs=1) as cpool,
            tc.tile_pool(name="xu", bufs=2) as xupool,
            tc.tile_pool(name="qk", bufs=1) as qkpool,
            tc.tile_pool(name="vsb", bufs=1) as vpool,
            tc.tile_pool(name="esb", bufs=3) as epool,
            tc.tile_pool(name="stg", bufs=8) as spool,
            tc.tile_pool(name="gat", bufs=4) as gpool,
            tc.tile_pool(name="bsb", bufs=2) as bpool,
            tc.tile_pool(name="osb", bufs=3) as opool,
            tc.tile_pool(name="psS", bufs=2, space="PSUM") as psS,
            tc.tile_pool(name="psU", bufs=2, space="PSUM") as psU,
            tc.tile_pool(name="psQ", bufs=2, space="PSUM") as psQ,
        ):
            # ---- constants / weights (resident) ----
            wqkv = cpool.tile([128, KT, D3], BF16, tag="wqkv")
            nc.gpsimd.dma_start(
                wqkv[:], wqkv_d.ap().rearrange("(ko p) n -> p ko n", p=128)
            )
            wproj = cpool.tile([128, KT, D], BF16, tag="wproj")
            nc.gpsimd.dma_start(
                wproj[:], wproj_d.ap().rearrange("(ko p) n -> p ko n", p=128)
            )
            bqk = cpool.tile([128, 12], F32, tag="bqk")
            nc.sync.dma_start(bqk[:], bqk_d.ap())
            bv1 = cpool.tile([1, D], BF16, tag="bv1")
            nc.gpsimd.dma_start(bv1[:], bv_d.ap())
            bvb = cpool.tile([128, D], BF16, tag="bvb")
            nc.gpsimd.partition_broadcast(bvb[:], bv1[:])
            bproj1 = cpool.tile([1, D], BF16, tag="bproj1")
            nc.gpsimd.dma_start(bproj1[:], bproj_d.ap())
            bprojb = cpool.tile([128, D], BF16, tag="bprojb")
            nc.gpsimd.partition_broadcast(bprojb[:], bproj1[:])

            qT = qkpool.tile([128, NPAIR, TN], BF16, tag="qT")
            kT = qkpool.tile([128, NPAIR, TN], BF16, tag="kT")
            vsb = vpool.tile([128, NJT, H, HD + 1], BF16, tag="v")
            # ones columns of v_ext (col 64 of every (jt, h) slot)
            nc.gpsimd.dma_start(
                vsb[:, :, :, HD : HD + 1],
                ones_d.ap().rearrange("p (a b) -> p a b", a=NJT),
            )

            for b in range(NB):
                tok0 = b * TN
                # ---- stage A: load xT, compute Q^T / K^T / V ----
                xTb = xupool.tile([128, KT, TN], BF16, tag="x", name=f"xT{b}")
                nc.gpsimd.dma_start(
                    xTb[:],
                    xT_d.ap().rearrange("(ko p) n -> p ko n", p=128)[
                        :, :, tok0 : tok0 + TN
                    ],
                )

                # Q^T and K^T: feature-major. m-tile m holds heads (2m, 2m+1)
                # for Q (m<6) or K (m-6).
                for m in range(12):
                    dst = qT if m < 6 else kT
                    hp = m % 6
                    for ih in range(2):
                        ps = psQ.tile([128, 512], F32, tag="ps")
                        for k in range(KT):
                            nc.tensor.matmul(
                                ps[:],
                                wqkv[:, k, m * 128 : (m + 1) * 128],
                                xTb[:, k, ih * 512 : (ih + 1) * 512],
                                start=(k == 0),
                                stop=(k == KT - 1),
                            )
                        nc.vector.tensor_scalar_add(
                            dst[:, hp, ih * 512 : (ih + 1) * 512],
                            ps[:],
                            bqk[:, m : m + 1],
                        )

                # V token-major, stored bf16 into v_ext slots (no bias here;
                # b_v is added after normalization, where it is per-partition)
                for t in range(NJT):
                    for nh in range(2):
                        ps = psQ.tile([128, 384], F32, tag="ps")
                        for k in range(KT):
                            nc.tensor.matmul(
                                ps[:],
                                xTb[:, k, t * 128 : (t + 1) * 128],
                                wqkv[:, k, 2 * D + nh * 384 : 2 * D + (nh + 1) * 384],
                                start=(k == 0),
                                stop=(k == KT - 1),
                            )
                        # [128, 384] -> heads nh*6..nh*6+5, cols 0:64 of
                        # v_ext, + b_v (b_v then flows through exp@v_ext and
                        # the ones-column normalization exactly)
                        nc.vector.tensor_add(
                            vsb[:, t, nh * 6 : (nh + 1) * 6, 0:HD],
                            ps[:],
                            bvb[:, nh * 384 : (nh + 1) * 384],
                        )

                # ---- stage B: attention, per head pair, per i-half ----
                uT = xupool.tile([128, KT, TN], BF16, tag="u", name=f"uT{b}")

                def emit_norm(hp, stages):
                    """Normalize one head-pair: batched 4-row reciprocal at
                    partition bases 0/32/64/96 (one ~6cpe reciprocal for all
                    four U tiles), gpsimd gathers/broadcasts, DVE multiplies.
                    Runs deferred — off TensorE's critical path."""
                    g = gpool.tile([97, 512], F32, tag="g")
                    nc.gpsimd.memset(g[:], 1.0)
                    for (ih, h, ust) in stages:
                        nc.gpsimd.tensor_copy(
                            g[32 * (2 * ih + h) : 32 * (2 * ih + h) + 1, :],
                            ust[HD : HD + 1, :],
                        )
                    rc = gpool.tile([97, 512], F32, tag="g")
                    nc.vector.reciprocal(rc[:], g[:])
                    # partition_broadcast reads the tile's absolute partition
                    # 0 — shift rows 32/64/96 down into dead tiles first.
                    shifted = {0: rc}
                    for idx in (1, 2, 3):
                        t = gpool.tile([97, 512], F32, tag="gs", name=f"gs{idx}")
                        nc.vector.tensor_copy(
                            t[0:1, :], rc[32 * idx : 32 * idx + 1, :]
                        )
                        shifted[idx] = t
                    for (ih, h, ust) in stages:
                        idx = 2 * ih + h
                        rb = bpool.tile([128, 512], F32, tag="rb")
                        nc.gpsimd.partition_broadcast(rb[:], shifted[idx][0:1, :])
                        usl = uT[
                            h * 64 : (h + 1) * 64, hp, ih * 512 : ih * 512 + 512
                        ]
                        nc.vector.tensor_mul(usl, ust[0:HD, :], rb[0:HD, :])

                pending = None
                for hp in range(NPAIR):
                    stages = []
                    for ih in range(2):
                        i0 = ih * 512
                        pu = [
                            psU.tile([HD + 1, 512], F32, tag="pu", name=f"pu{h}")
                            for h in range(2)
                        ]
                        for jt in range(NJT):
                            # scores^T for both heads of the pair, row-packed
                            ps = psS.tile([128, 1024], F32, tag="s")
                            for h in range(2):
                                nc.tensor.matmul(
                                    ps[:, h * 512 : (h + 1) * 512],
                                    kT[
                                        h * 64 : (h + 1) * 64,
                                        hp,
                                        jt * 128 : (jt + 1) * 128,
                                    ],
                                    qT[h * 64 : (h + 1) * 64, hp, i0 : i0 + 512],
                                )
                            e = epool.tile([128, 1024], BF16, tag="e")
                            nc.scalar.activation(e[:], ps[:], AF.Exp, scale=SCALE)
                            for h in range(2):
                                nc.tensor.matmul(
                                    pu[h][:],
                                    vsb[:, jt, 2 * hp + h, :],
                                    e[:, h * 512 : (h + 1) * 512],
                                    start=(jt == 0),
                                    stop=(jt == NJT - 1),
                                )
                        # Evacuate U+r fast: ONLY these two copies gate
                        # PSUM release (keeps TensorE dense and HAM warm).
                        # Normalization is deferred one pair so the slow
                        # reciprocal never sits ahead of the next stage
                        # copies in DVE's in-order queue.
                        for h in range(2):
                            ust = spool.tile([HD + 1, 512], F32, tag="ust")
                            nc.vector.tensor_copy(ust[:], pu[h][:])
                            stages.append((ih, h, ust))
                    if pending is not None:
                        emit_norm(*pending)
                    pending = (hp, stages)
                if pending is not None:
                    emit_norm(*pending)
                    pending = None

                # ---- stage P: output projection ----
                for t in range(NJT):
                    for nh in range(2):
                        ps = psQ.tile([128, 384], F32, tag="ps")
                        for k in range(KT):
                            nc.tensor.matmul(
                                ps[:],
                                uT[:, k, t * 128 : (t + 1) * 128],
                                wproj[:, k, nh * 384 : (nh + 1) * 384],
                                start=(k == 0),
                                stop=(k == KT - 1),
                            )
                        ot = opool.tile([128, 384], F32, tag="o")
                        nc.vector.tensor_add(
                            ot[:],
                            ps[:],
                            bprojb[:, nh * 384 : (nh + 1) * 384],
                        )
                        nc.sync.dma_start(
                            out_d.ap()[
                                tok0 + t * 128 : tok0 + (t + 1) * 128,
                                nh * 384 : (nh + 1) * 384,
                            ],
                            ot[:],
                        )

    nc.compile()
    return nc


_NC_CACHE = None


def _get_nc():
    global _NC_CACHE
    if _NC_CACHE is None:
        _NC_CACHE = build()
    return _NC_CACHE


def _prep_core_inputs(x_c, w_qkv, b_qkv, w_proj, b_proj):
    """Host-side layout prep for one core. x_c: [2, 1024, 768]."""
    xT = np.ascontiguousarray(x_c.reshape(T, D).T).astype(np.float32)
    bqk = np.ascontiguousarray(b_qkv[: 12 * 128].reshape(12, 128).T)
    return {
        "xT": xT,
        "wqkv": np.ascontiguousarray(w_qkv, dtype=np.float32),
        "wproj": np.ascontiguousarray(w_proj, dtype=np.float32),
        "bqk": bqk.astype(np.float32),
        "bv": np.ascontiguousarray(b_qkv[2 * D :].reshape(1, D), dtype=np.float32),
        "bproj": np.ascontiguousarray(b_proj.reshape(1, D), dtype=np.float32),
        "ones": np.ones((128, 96), dtype=np.float32),
    }


def kernel(x, w_qkv, b_qkv, w_proj, b_proj):
    x = np.asarray(x, dtype=np.float32)
    w_qkv = np.asarray(w_qkv, dtype=np.float32)
    b_qkv = np.asarray(b_qkv, dtype=np.float32)
    w_proj = np.asarray(w_proj, dtype=np.float32)
    b_proj = np.asarray(b_proj, dtype=np.float32)
    B, N, Dd = x.shape
    assert (B, N, Dd) == (16, 1024, 768)

    nc = _get_nc()
    in_maps = [
        _prep_core_inputs(x[2 * c : 2 * c + 2], w_qkv, b_qkv, w_proj, b_proj)
        for c in range(8)
    ]
    res = run_bass_kernel_spmd(nc, in_maps, core_ids=list(range(8)))
    out = np.empty((B, N, Dd), dtype=np.float32)
    for c in range(8):
        out[2 * c : 2 * c + 2] = res.results[c]["out"].reshape(2, N, Dd)
    kernel.last_results = res
    return out


# revision 13
# speedup vs baseline: 1.2704x; 1.0616x over previous
"""Multi-head attention kernel for Trainium2, data-parallel over 8 NeuronCores.

Problem: B=16, N=1024, D=768, H=12 heads (hd=64), fp32 I/O.
  qkv = x @ w_qkv + b_qkv ; attention ; out = attn_out @ w_proj + b_proj

Sharding: batch data-parallel — core c handles batches [2c, 2c+2); weights
replicated. Inside each core, the two batches are processed sequentially.

Layout strategy (all compute in f32r on TensorE — tf32-like, ~1.6e-4 rel):
  - host pre-transposes x to xT [768, T] so the in-feature contraction has
    features on partitions for both operands.
  - Q^T, K^T computed feature-major [768, N]: lhsT = w_qkv cols, rhs = xT.
    A 128-row feature tile holds a PAIR of heads (2x64) -> scores matmuls
    for the two heads run concurrently via tile_position row packing (K=64).
  - V computed token-major [N, 768]: lhsT = xT chunk, rhs = w_qkv v-cols,
    stored bf16 with a ones column appended per head (v_ext [128, 65]).
  - scores^T tiles [128 j, 512 q] per head -> one ACT exp op [128, 1024]
    covers both heads of a pair (softmax scale folded into exp's scale).
  - U^T = sum_j exp * v_ext accumulates in PSUM [65, 512]; row 64 is the
    softmax denominator (ones column) — no separate reduction needed.
  - normalize: reciprocal (DVE) -> partition_broadcast (GpSimd) -> multiply
    (DVE), + b_v per-partition. b_q/b_k added at Q^T/K^T evacuation
    (per-partition in feature-major layout).
  - proj: lhsT = attn^T tile, rhs = w_proj; + b_proj via broadcast add.
    Output lands token-major [T, 768] == final layout.
"""

import contextlib
import ctypes
import os
import sys
import types

import numpy as np

# ---------------------------------------------------------------------------
# NTFF profiling shim: bass_utils's trace path imports
# antenv.axon_hooks.get_axon_ntff_profile_hook, which this container's antenv
# lacks. Register a ctypes-based equivalent so BASS_TRACE=1 works. Harmless
# if tracing is never requested.
# ---------------------------------------------------------------------------


def _install_ntff_shim():
    if "antenv.axon_hooks" in sys.modules:
        return
    so_path = "/opt/axon/libaxon_pjrt.so"
    hook = None
    try:
        lib = ctypes.CDLL(so_path)
        if hasattr(lib, "axon_start_nrt_profile"):
            lib.axon_start_nrt_profile.argtypes = [
                ctypes.POINTER(ctypes.c_int64),
                ctypes.c_size_t,
            ]
            lib.axon_start_nrt_profile.restype = ctypes.c_int64
            lib.axon_stop_nrt_profile.argtypes = [ctypes.c_char_p]
            lib.axon_stop_nrt_profile.restype = ctypes.c_int64

            @contextlib.contextmanager
            def _hook(output_dir, device_ids):
                import jax

                jax.devices()
                if device_ids:
                    ids = (ctypes.c_int64 * len(device_ids))(*device_ids)
                    rc = lib.axon_start_nrt_profile(ids, len(device_ids))
                else:
                    rc = lib.axon_start_nrt_profile(None, 0)
                if rc != 0:
                    raise RuntimeError(f"axon_start_nrt_profile rc={rc}")
                try:
                    yield
                finally:
                    n = lib.axon_stop_nrt_profile(str(output_dir).encode())
                    print(f"ntff profile: {n} file(s) in {output_dir}",
                          file=sys.stderr)

            hook = _hook
    except OSError:
        pass
    mod = types.ModuleType("antenv.axon_hooks")
    mod.get_axon_ntff_profile_hook = lambda: hook
    mod.set_axon_ntff_profile_hook = lambda h: None
    sys.modules["antenv.axon_hooks"] = mod


_install_ntff_shim()

import concourse.bacc as bacc  # noqa: E402
import concourse.mybir as mybir  # noqa: E402
import concourse.tile as tile  # noqa: E402
from concourse.bass_utils import run_bass_kernel_spmd  # noqa: E402

F32 = mybir.dt.float32
F32R = mybir.dt.float32r
BF16 = mybir.dt.bfloat16
AF = mybir.ActivationFunctionType

# Problem constants (per core)
NB = 2        # batches per core
TN = 1024     # tokens per batch
T = NB * TN   # tokens per core
D = 768
H = 12
HD = 64
D3 = 3 * D
KT = D // 128          # 6 contraction tiles
NPAIR = H // 2         # 6 head pairs
NJT = TN // 128        # 8 key tiles per batch
SCALE = HD ** -0.5


def build():
    nc = bacc.Bacc(None)
    xT_d = nc.declare_dram_parameter("xT", [D, T], F32, isOutput=False)
    wqkv_d = nc.declare_dram_parameter("wqkv", [D, D3], F32, isOutput=False)
    wproj_d = nc.declare_dram_parameter("wproj", [D, D], F32, isOutput=False)
    bqk_d = nc.declare_dram_parameter("bqk", [128, 12], F32, isOutput=False)
    bv_d = nc.declare_dram_parameter("bv", [1, D], F32, isOutput=False)
    bproj_d = nc.declare_dram_parameter("bproj", [1, D], F32, isOutput=False)
    ones_d = nc.declare_dram_parameter("ones", [128, 96], F32, isOutput=False)
    out_d = nc.declare_dram_parameter("out", [T, D], F32, isOutput=True)

    with tile.TileContext(nc) as tc:
        with (
            nc.allow_low_precision(reason="f32r/bf16 attention pipeline"),
            tc.tile_pool(name="const", bufs=1) as cpool,
            tc.tile_pool(name="xu", bufs=2) as xupool,
            tc.tile_pool(name="qk", bufs=2) as qkpool,
            tc.tile_pool(name="vsb", bufs=2) as vpool,
            tc.tile_pool(name="esb", bufs=3) as epool,
            tc.tile_pool(name="stg", bufs=10) as spool,
            tc.tile_pool(name="gat", bufs=2) as gpool,
            tc.tile_pool(name="gsh", bufs=3) as gspool,
            tc.tile_pool(name="bsb", bufs=2) as bpool,
            tc.tile_pool(name="osb", bufs=3) as opool,
            tc.tile_pool(name="psS", bufs=2, space="PSUM") as psS,
            tc.tile_pool(name="psU", bufs=2, space="PSUM") as psU,
            tc.tile_pool(name="psQ", bufs=2, space="PSUM") as psQ,
        ):
            # ---- constants / weights (resident) ----
            wqkv = cpool.tile([128, KT, D3], BF16, tag="wqkv")
            nc.gpsimd.dma_start(
                wqkv[:], wqkv_d.ap().rearrange("(ko p) n -> p ko n", p=128)
            )
            wproj = cpool.tile([128, KT, D], BF16, tag="wproj")
            nc.gpsimd.dma_start(
                wproj[:], wproj_d.ap().rearrange("(ko p) n -> p ko n", p=128)
            )
            bqk = cpool.tile([128, 12], F32, tag="bqk")
            nc.sync.dma_start(bqk[:], bqk_d.ap())
            bv1 = cpool.tile([1, D], BF16, tag="bv1")
            nc.gpsimd.dma_start(bv1[:], bv_d.ap())
            bvb = cpool.tile([128, D], BF16, tag="bvb")
            nc.gpsimd.partition_broadcast(bvb[:], bv1[:])
            bproj1 = cpool.tile([1, D], BF16, tag="bproj1")
            nc.gpsimd.dma_start(bproj1[:], bproj_d.ap())
            bprojb = cpool.tile([128, D], BF16, tag="bprojb")
            nc.gpsimd.partition_broadcast(bprojb[:], bproj1[:])

            for b in range(NB):
                tok0 = b * TN
                qT = qkpool.tile([128, NPAIR, TN], BF16, tag="qT")
                kT = qkpool.tile([128, NPAIR, TN], BF16, tag="kT")
                vsb = vpool.tile([128, NJT, H, HD + 1], BF16, tag="v")
                # ones columns of v_ext (col 64 of every (jt, h) slot)
                nc.gpsimd.dma_start(
                    vsb[:, :, :, HD : HD + 1],
                    ones_d.ap().rearrange("p (a b) -> p a b", a=NJT),
                )
                # ---- stage A: load xT, compute Q^T / K^T / V ----
                xTb = xupool.tile([128, KT, TN], BF16, tag="x", name=f"xT{b}")
                nc.gpsimd.dma_start(
                    xTb[:],
                    xT_d.ap().rearrange("(ko p) n -> p ko n", p=128)[
                        :, :, tok0 : tok0 + TN
                    ],
                )

                # Q^T and K^T: feature-major. m-tile m holds heads (2m, 2m+1)
                # for Q (m<6) or K (m-6).
                for m in range(12):
                    dst = qT if m < 6 else kT
                    hp = m % 6
                    for ih in range(2):
                        ps = psQ.tile([128, 512], F32, tag="ps")
                        for k in range(KT):
                            nc.tensor.matmul(
                                ps[:],
                                wqkv[:, k, m * 128 : (m + 1) * 128],
                                xTb[:, k, ih * 512 : (ih + 1) * 512],
                                start=(k == 0),
                                stop=(k == KT - 1),
                            )
                        nc.vector.tensor_scalar_add(
                            dst[:, hp, ih * 512 : (ih + 1) * 512],
                            ps[:],
                            bqk[:, m : m + 1],
                        )

                # V token-major, stored bf16 into v_ext slots (no bias here;
                # b_v is added after normalization, where it is per-partition)
                for t in range(NJT):
                    for nh in range(2):
                        ps = psQ.tile([128, 384], F32, tag="ps")
                        for k in range(KT):
                            nc.tensor.matmul(
                                ps[:],
                                xTb[:, k, t * 128 : (t + 1) * 128],
                                wqkv[:, k, 2 * D + nh * 384 : 2 * D + (nh + 1) * 384],
                                start=(k == 0),
                                stop=(k == KT - 1),
                            )
                        # [128, 384] -> heads nh*6..nh*6+5, cols 0:64 of
                        # v_ext, + b_v (b_v then flows through exp@v_ext and
                        # the ones-column normalization exactly)
                        nc.vector.tensor_add(
                            vsb[:, t, nh * 6 : (nh + 1) * 6, 0:HD],
                            ps[:],
                            bvb[:, nh * 384 : (nh + 1) * 384],
                        )

                # ---- stage B: attention, per head pair, per i-half ----
                uT = xupool.tile([128, KT, TN], BF16, tag="u", name=f"uT{b}")

                def emit_norm(hp, stages):
                    """Normalize one head-pair: batched 4-row reciprocal at
                    partition bases 0/32/64/96 (one ~6cpe reciprocal for all
                    four U tiles), gpsimd gathers/broadcasts, DVE multiplies.
                    Runs deferred — off TensorE's critical path."""
                    g = gpool.tile([97, 512], F32, tag="g")
                    nc.vector.memset(g[:], 1.0)
                    for (ih, h, ust) in stages:
                        nc.vector.tensor_copy(
                            g[32 * (2 * ih + h) : 32 * (2 * ih + h) + 1, :],
                            ust[HD : HD + 1, :],
                        )
                    rc = gpool.tile([97, 512], F32, tag="g")
                    nc.vector.reciprocal(rc[:], g[:])
                    # partition_broadcast reads the tile's absolute partition
                    # 0 — shift rows 32/64/96 down into dead tiles first.
                    shifted = {0: rc}
                    for idx in (1, 2, 3):
                        t = gspool.tile([1, 512], F32, tag="gs", name=f"gs{idx}")
                        nc.vector.tensor_copy(
                            t[0:1, :], rc[32 * idx : 32 * idx + 1, :]
                        )
                        shifted[idx] = t
                    for (ih, h, ust) in stages:
                        idx = 2 * ih + h
                        rb = bpool.tile([128, 512], F32, tag="rb")
                        nc.gpsimd.partition_broadcast(rb[:], shifted[idx][0:1, :])
                        usl = uT[
                            h * 64 : (h + 1) * 64, hp, ih * 512 : ih * 512 + 512
                        ]
                        nc.vector.tensor_mul(usl, ust[0:HD, :], rb[0:HD, :])

                pending = None
                for hp in range(NPAIR):
                    stages = []
                    for ih in range(2):
                        i0 = ih * 512
                        pu = [
                            psU.tile([HD + 1, 512], F32, tag="pu", name=f"pu{h}")
                            for h in range(2)
                        ]
                        for jt in range(NJT):
                            # scores^T for both heads of the pair, row-packed
                            ps = psS.tile([128, 1024], F32, tag="s")
                            for h in range(2):
                                nc.tensor.matmul(
                                    ps[:, h * 512 : (h + 1) * 512],
                                    kT[
                                        h * 64 : (h + 1) * 64,
                                        hp,
                                        jt * 128 : (jt + 1) * 128,
                                    ],
                                    qT[h * 64 : (h + 1) * 64, hp, i0 : i0 + 512],
                                )
                            e = epool.tile([128, 1024], BF16, tag="e")
                            nc.scalar.activation(e[:], ps[:], AF.Exp, scale=SCALE)
                            for h in range(2):
                                nc.tensor.matmul(
                                    pu[h][:],
                                    vsb[:, jt, 2 * hp + h, :],
                                    e[:, h * 512 : (h + 1) * 512],
                                    start=(jt == 0),
                                    stop=(jt == NJT - 1),
                                )
                        # Evacuate U+r fast: ONLY these two copies gate
                        # PSUM release (keeps TensorE dense and HAM warm).
                        # Normalization is deferred one pair so the slow
                        # reciprocal never sits ahead of the next stage
                        # copies in DVE's in-order queue.
                        for h in range(2):
                            ust = spool.tile([HD + 1, 512], F32, tag="ust")
                            nc.vector.tensor_copy(ust[:], pu[h][:])
                            stages.append((ih, h, ust))
                    if pending is not None:
                        emit_norm(*pending)
                    pending = (hp, stages)
                if pending is not None:
                    emit_norm(*pending)
                    pending = None

                # ---- stage P: output projection ----
                for t in range(NJT):
                    for nh in range(2):
                        ps = psQ.tile([128, 384], F32, tag="ps")
                        for k in range(KT):
                            nc.tensor.matmul(
                                ps[:],
                                uT[:, k, t * 128 : (t + 1) * 128],
                                wproj[:, k, nh * 384 : (nh + 1) * 384],
                                start=(k == 0),
                                stop=(k == KT - 1),
                            )
                        ot = opool.tile([128, 384], F32, tag="o")
                        nc.vector.tensor_add(
                            ot[:],
                            ps[:],
                            bprojb[:, nh * 384 : (nh + 1) * 384],
                        )
                        nc.sync.dma_start(
                            out_d.ap()[
                                tok0 + t * 128 : tok0 + (t + 1) * 128,
                                nh * 384 : (nh + 1) * 384,
                            ],
                            ot[:],
                        )

    nc.compile()
    return nc


_NC_CACHE = None


def _get_nc():
    global _NC_CACHE
    if _NC_CACHE is None:
        _NC_CACHE = build()
    return _NC_CACHE


def _prep_core_inputs(x_c, w_qkv, b_qkv, w_proj, b_proj):
    """Host-side layout prep for one core. x_c: [2, 1024, 768]."""
    xT = np.ascontiguousarray(x_c.reshape(T, D).T).astype(np.float32)
    bqk = np.ascontiguousarray(b_qkv[: 12 * 128].reshape(12, 128).T)
    return {
        "xT": xT,
        "wqkv": np.ascontiguousarray(w_qkv, dtype=np.float32),
        "wproj": np.ascontiguousarray(w_proj, dtype=np.float32),
        "bqk": bqk.astype(np.float32),
        "bv": np.ascontiguousarray(b_qkv[2 * D :].reshape(1, D), dtype=np.float32),
        "bproj": np.ascontiguousarray(b_proj.reshape(1, D), dtype=np.float32),
        "ones": np.ones((128, 96), dtype=np.float32),
    }


def kernel(x, w_qkv, b_qkv, w_proj, b_proj):
    x = np.asarray(x, dtype=np.float32)
    w_qkv = np.asarray(w_qkv, dtype=np.float32)
    b_qkv = np.asarray(b_qkv, dtype=np.float32)
    w_proj = np.asarray(w_proj, dtype=np.float32)
    b_proj = np.asarray(b_proj, dtype=np.float32)
    B, N, Dd = x.shape
    assert (B, N, Dd) == (16, 1024, 768)

    nc = _get_nc()
    in_maps = [
        _prep_core_inputs(x[2 * c : 2 * c + 2], w_qkv, b_qkv, w_proj, b_proj)
        for c in range(8)
    ]
    res = run_bass_kernel_spmd(nc, in_maps, core_ids=list(range(8)))
    out = np.empty((B, N, Dd), dtype=np.float32)
    for c in range(8):
        out[2 * c : 2 * c + 2] = res.results[c]["out"].reshape(2, N, Dd)
    kernel.last_results = res
    return out


# revision 14
# speedup vs baseline: 1.2869x; 1.0130x over previous
"""Multi-head attention kernel for Trainium2, data-parallel over 8 NeuronCores.

Problem: B=16, N=1024, D=768, H=12 heads (hd=64), fp32 I/O.
  qkv = x @ w_qkv + b_qkv ; attention ; out = attn_out @ w_proj + b_proj

Sharding: batch data-parallel — core c handles batches [2c, 2c+2); weights
replicated. Inside each core, the two batches are processed sequentially.

Layout strategy (all compute in f32r on TensorE — tf32-like, ~1.6e-4 rel):
  - host pre-transposes x to xT [768, T] so the in-feature contraction has
    features on partitions for both operands.
  - Q^T, K^T computed feature-major [768, N]: lhsT = w_qkv cols, rhs = xT.
    A 128-row feature tile holds a PAIR of heads (2x64) -> scores matmuls
    for the two heads run concurrently via tile_position row packing (K=64).
  - V computed token-major [N, 768]: lhsT = xT chunk, rhs = w_qkv v-cols,
    stored bf16 with a ones column appended per head (v_ext [128, 65]).
  - scores^T tiles [128 j, 512 q] per head -> one ACT exp op [128, 1024]
    covers both heads of a pair (softmax scale folded into exp's scale).
  - U^T = sum_j exp * v_ext accumulates in PSUM [65, 512]; row 64 is the
    softmax denominator (ones column) — no separate reduction needed.
  - normalize: reciprocal (DVE) -> partition_broadcast (GpSimd) -> multiply
    (DVE), + b_v per-partition. b_q/b_k added at Q^T/K^T evacuation
    (per-partition in feature-major layout).
  - proj: lhsT = attn^T tile, rhs = w_proj; + b_proj via broadcast add.
    Output lands token-major [T, 768] == final layout.
"""

import contextlib
import ctypes
import os
import sys
import types

import numpy as np

# ---------------------------------------------------------------------------
# NTFF profiling shim: bass_utils's trace path imports
# antenv.axon_hooks.get_axon_ntff_profile_hook, which this container's antenv
# lacks. Register a ctypes-based equivalent so BASS_TRACE=1 works. Harmless
# if tracing is never requested.
# ---------------------------------------------------------------------------


def _install_ntff_shim():
    if "antenv.axon_hooks" in sys.modules:
        return
    so_path = "/opt/axon/libaxon_pjrt.so"
    hook = None
    try:
        lib = ctypes.CDLL(so_path)
        if hasattr(lib, "axon_start_nrt_profile"):
            lib.axon_start_nrt_profile.argtypes = [
                ctypes.POINTER(ctypes.c_int64),
                ctypes.c_size_t,
            ]
            lib.axon_start_nrt_profile.restype = ctypes.c_int64
            lib.axon_stop_nrt_profile.argtypes = [ctypes.c_char_p]
            lib.axon_stop_nrt_profile.restype = ctypes.c_int64

            @contextlib.contextmanager
            def _hook(output_dir, device_ids):
                import jax

                jax.devices()
                if device_ids:
                    ids = (ctypes.c_int64 * len(device_ids))(*device_ids)
                    rc = lib.axon_start_nrt_profile(ids, len(device_ids))
                else:
                    rc = lib.axon_start_nrt_profile(None, 0)
                if rc != 0:
                    raise RuntimeError(f"axon_start_nrt_profile rc={rc}")
                try:
                    yield
                finally:
                    n = lib.axon_stop_nrt_profile(str(output_dir).encode())
                    print(f"ntff profile: {n} file(s) in {output_dir}",
                          file=sys.stderr)

            hook = _hook
    except OSError:
        pass
    mod = types.ModuleType("antenv.axon_hooks")
    mod.get_axon_ntff_profile_hook = lambda: hook
    mod.set_axon_ntff_profile_hook = lambda h: None
    sys.modules["antenv.axon_hooks"] = mod


_install_ntff_shim()

import concourse.bacc as bacc  # noqa: E402
import concourse.mybir as mybir  # noqa: E402
import concourse.tile as tile  # noqa: E402
from concourse.bass_utils import run_bass_kernel_spmd  # noqa: E402

F32 = mybir.dt.float32
F32R = mybir.dt.float32r
BF16 = mybir.dt.bfloat16
AF = mybir.ActivationFunctionType

# Problem constants (per core)
NB = 2        # batches per core
TN = 1024     # tokens per batch
T = NB * TN   # tokens per core
D = 768
H = 12
HD = 64
D3 = 3 * D
KT = D // 128          # 6 contraction tiles
NPAIR = H // 2         # 6 head pairs
NJT = TN // 128        # 8 key tiles per batch
SCALE = HD ** -0.5


def build():
    nc = bacc.Bacc(None)
    xT_d = nc.declare_dram_parameter("xT", [D, T], BF16, isOutput=False)
    wqkv_d = nc.declare_dram_parameter("wqkv", [D, D3], BF16, isOutput=False)
    wproj_d = nc.declare_dram_parameter("wproj", [D, D], BF16, isOutput=False)
    bqk_d = nc.declare_dram_parameter("bqk", [128, 12], F32, isOutput=False)
    bv_d = nc.declare_dram_parameter("bv", [1, D], BF16, isOutput=False)
    bproj_d = nc.declare_dram_parameter("bproj", [1, D], BF16, isOutput=False)
    ones_d = nc.declare_dram_parameter("ones", [128, 96], BF16, isOutput=False)
    out_d = nc.declare_dram_parameter("out", [T, D], F32, isOutput=True)

    with tile.TileContext(nc) as tc:
        with (
            nc.allow_low_precision(reason="f32r/bf16 attention pipeline"),
            tc.tile_pool(name="const", bufs=1) as cpool,
            tc.tile_pool(name="xu", bufs=2) as xupool,
            tc.tile_pool(name="qk", bufs=2) as qkpool,
            tc.tile_pool(name="vsb", bufs=2) as vpool,
            tc.tile_pool(name="esb", bufs=3) as epool,
            tc.tile_pool(name="stg", bufs=10) as spool,
            tc.tile_pool(name="gat", bufs=2) as gpool,
            tc.tile_pool(name="gsh", bufs=3) as gspool,
            tc.tile_pool(name="bsb", bufs=2) as bpool,
            tc.tile_pool(name="osb", bufs=3) as opool,
            tc.tile_pool(name="psS", bufs=2, space="PSUM") as psS,
            tc.tile_pool(name="psU", bufs=2, space="PSUM") as psU,
            tc.tile_pool(name="psQ", bufs=2, space="PSUM") as psQ,
        ):
            # ---- constants / weights (resident) ----
            wqkv = cpool.tile([128, KT, D3], BF16, tag="wqkv")
            nc.sync.dma_start(
                wqkv[:], wqkv_d.ap().rearrange("(ko p) n -> p ko n", p=128)
            )
            wproj = cpool.tile([128, KT, D], BF16, tag="wproj")
            nc.sync.dma_start(
                wproj[:], wproj_d.ap().rearrange("(ko p) n -> p ko n", p=128)
            )
            bqk = cpool.tile([128, 12], F32, tag="bqk")
            nc.sync.dma_start(bqk[:], bqk_d.ap())
            bv1 = cpool.tile([1, D], BF16, tag="bv1")
            nc.sync.dma_start(bv1[:], bv_d.ap())
            bvb = cpool.tile([128, D], BF16, tag="bvb")
            nc.gpsimd.partition_broadcast(bvb[:], bv1[:])
            bproj1 = cpool.tile([1, D], BF16, tag="bproj1")
            nc.sync.dma_start(bproj1[:], bproj_d.ap())
            bprojb = cpool.tile([128, D], BF16, tag="bprojb")
            nc.gpsimd.partition_broadcast(bprojb[:], bproj1[:])

            for b in range(NB):
                tok0 = b * TN
                qT = qkpool.tile([128, NPAIR, TN], BF16, tag="qT")
                kT = qkpool.tile([128, NPAIR, TN], BF16, tag="kT")
                vsb = vpool.tile([128, NJT, H, HD + 1], BF16, tag="v")
                # ones columns of v_ext (col 64 of every (jt, h) slot)
                nc.sync.dma_start(
                    vsb[:, :, :, HD : HD + 1],
                    ones_d.ap().rearrange("p (a b) -> p a b", a=NJT),
                )
                # ---- stage A: load xT, compute Q^T / K^T / V ----
                xTb = xupool.tile([128, KT, TN], BF16, tag="x", name=f"xT{b}")
                nc.sync.dma_start(
                    xTb[:],
                    xT_d.ap().rearrange("(ko p) n -> p ko n", p=128)[
                        :, :, tok0 : tok0 + TN
                    ],
                )

                # Q^T and K^T: feature-major. m-tile m holds heads (2m, 2m+1)
                # for Q (m<6) or K (m-6).
                for m in [0, 6, 1, 7, 2, 8, 3, 9, 4, 10, 5, 11]:
                    dst = qT if m < 6 else kT
                    hp = m % 6
                    for ih in range(2):
                        ps = psQ.tile([128, 512], F32, tag="ps")
                        for k in range(KT):
                            nc.tensor.matmul(
                                ps[:],
                                wqkv[:, k, m * 128 : (m + 1) * 128],
                                xTb[:, k, ih * 512 : (ih + 1) * 512],
                                start=(k == 0),
                                stop=(k == KT - 1),
                            )
                        nc.vector.tensor_scalar_add(
                            dst[:, hp, ih * 512 : (ih + 1) * 512],
                            ps[:],
                            bqk[:, m : m + 1],
                        )

                # V token-major, stored bf16 into v_ext slots (no bias here;
                # b_v is added after normalization, where it is per-partition)
                for t in range(NJT):
                    for nh in range(2):
                        ps = psQ.tile([128, 384], F32, tag="ps")
                        for k in range(KT):
                            nc.tensor.matmul(
                                ps[:],
                                xTb[:, k, t * 128 : (t + 1) * 128],
                                wqkv[:, k, 2 * D + nh * 384 : 2 * D + (nh + 1) * 384],
                                start=(k == 0),
                                stop=(k == KT - 1),
                            )
                        # [128, 384] -> heads nh*6..nh*6+5, cols 0:64 of
                        # v_ext, + b_v (b_v then flows through exp@v_ext and
                        # the ones-column normalization exactly)
                        nc.vector.tensor_add(
                            vsb[:, t, nh * 6 : (nh + 1) * 6, 0:HD],
                            ps[:],
                            bvb[:, nh * 384 : (nh + 1) * 384],
                        )

                # ---- stage B: attention, per head pair, per i-half ----
                uT = xupool.tile([128, KT, TN], BF16, tag="u", name=f"uT{b}")

                def emit_norm(hp, stages):
                    """Normalize one head-pair: batched 4-row reciprocal at
                    partition bases 0/32/64/96 (one ~6cpe reciprocal for all
                    four U tiles), gpsimd gathers/broadcasts, DVE multiplies.
                    Runs deferred — off TensorE's critical path."""
                    g = gpool.tile([97, 512], F32, tag="g")
                    nc.vector.memset(g[:], 1.0)
                    for (ih, h, ust) in stages:
                        nc.vector.tensor_copy(
                            g[32 * (2 * ih + h) : 32 * (2 * ih + h) + 1, :],
                            ust[HD : HD + 1, :],
                        )
                    rc = gpool.tile([97, 512], F32, tag="g")
                    nc.vector.reciprocal(rc[:], g[:])
                    # partition_broadcast reads the tile's absolute partition
                    # 0 — shift rows 32/64/96 down into dead tiles first.
                    shifted = {0: rc}
                    for idx in (1, 2, 3):
                        t = gspool.tile([1, 512], F32, tag="gs", name=f"gs{idx}")
                        nc.vector.tensor_copy(
                            t[0:1, :], rc[32 * idx : 32 * idx + 1, :]
                        )
                        shifted[idx] = t
                    for (ih, h, ust) in stages:
                        idx = 2 * ih + h
                        rb = bpool.tile([128, 512], F32, tag="rb")
                        nc.gpsimd.partition_broadcast(rb[:], shifted[idx][0:1, :])
                        usl = uT[
                            h * 64 : (h + 1) * 64, hp, ih * 512 : ih * 512 + 512
                        ]
                        nc.vector.tensor_mul(usl, ust[0:HD, :], rb[0:HD, :])

                pending = None
                for hp in range(NPAIR):
                    stages = []
                    for ih in range(2):
                        i0 = ih * 512
                        pu = [
                            psU.tile([HD + 1, 512], F32, tag="pu", name=f"pu{h}")
                            for h in range(2)
                        ]
                        for jt in range(NJT):
                            # scores^T for both heads of the pair, row-packed
                            ps = psS.tile([128, 1024], F32, tag="s")
                            for h in range(2):
                                nc.tensor.matmul(
                                    ps[:, h * 512 : (h + 1) * 512],
                                    kT[
                                        h * 64 : (h + 1) * 64,
                                        hp,
                                        jt * 128 : (jt + 1) * 128,
                                    ],
                                    qT[h * 64 : (h + 1) * 64, hp, i0 : i0 + 512],
                                )
                            e = epool.tile([128, 1024], BF16, tag="e")
                            nc.scalar.activation(e[:], ps[:], AF.Exp, scale=SCALE)
                            for h in range(2):
                                nc.tensor.matmul(
                                    pu[h][:],
                                    vsb[:, jt, 2 * hp + h, :],
                                    e[:, h * 512 : (h + 1) * 512],
                                    start=(jt == 0),
                                    stop=(jt == NJT - 1),
                                )
                        # Evacuate U+r fast: ONLY these two copies gate
                        # PSUM release (keeps TensorE dense and HAM warm).
                        # Normalization is deferred one pair so the slow
                        # reciprocal never sits ahead of the next stage
                        # copies in DVE's in-order queue.
                        for h in range(2):
                            ust = spool.tile([HD + 1, 512], F32, tag="ust")
                            nc.vector.tensor_copy(ust[:], pu[h][:])
                            stages.append((ih, h, ust))
                    if pending is not None:
                        emit_norm(*pending)
                    pending = (hp, stages)
                if pending is not None:
                    emit_norm(*pending)
                    pending = None

                # ---- stage P: output projection ----
                for t in range(NJT):
                    for nh in range(2):
                        ps = psQ.tile([128, 384], F32, tag="ps")
                        for k in range(KT):
                            nc.tensor.matmul(
                                ps[:],
                                uT[:, k, t * 128 : (t + 1) * 128],
                                wproj[:, k, nh * 384 : (nh + 1) * 384],
                                start=(k == 0),
                                stop=(k == KT - 1),
                            )
                        ot = opool.tile([128, 384], F32, tag="o")
                        nc.vector.tensor_add(
                            ot[:],
                            ps[:],
                            bprojb[:, nh * 384 : (nh + 1) * 384],
                        )
                        nc.sync.dma_start(
                            out_d.ap()[
                                tok0 + t * 128 : tok0 + (t + 1) * 128,
                                nh * 384 : (nh + 1) * 384,
                            ],
                            ot[:],
                        )

    nc.compile()
    return nc


_NC_CACHE = None


def _get_nc():
    global _NC_CACHE
    if _NC_CACHE is None:
        _NC_CACHE = build()
    return _NC_CACHE


def _prep_core_inputs(x_c, w_qkv, b_qkv, w_proj, b_proj):
    """Host-side layout prep for one core. x_c: [2, 1024, 768]."""
    xT = np.ascontiguousarray(x_c.reshape(T, D).T).astype(np.float32)
    bqk = np.ascontiguousarray(b_qkv[: 12 * 128].reshape(12, 128).T)
    import ml_dtypes

    bf = ml_dtypes.bfloat16
    return {
        "xT": np.ascontiguousarray(xT.astype(bf)),
        "wqkv": np.ascontiguousarray(w_qkv.astype(bf)),
        "wproj": np.ascontiguousarray(w_proj.astype(bf)),
        "bqk": bqk.astype(np.float32),
        "bv": np.ascontiguousarray(b_qkv[2 * D :].reshape(1, D).astype(bf)),
        "bproj": np.ascontiguousarray(b_proj.reshape(1, D).astype(bf)),
        "ones": np.ones((128, 96), dtype=bf),
    }


def kernel(x, w_qkv, b_qkv, w_proj, b_proj):
    x = np.asarray(x, dtype=np.float32)
    w_qkv = np.asarray(w_qkv, dtype=np.float32)
    b_qkv = np.asarray(b_qkv, dtype=np.float32)
    w_proj = np.asarray(w_proj, dtype=np.float32)
    b_proj = np.asarray(b_proj, dtype=np.float32)
    B, N, Dd = x.shape
    assert (B, N, Dd) == (16, 1024, 768)

    nc = _get_nc()
    in_maps = [
        _prep_core_inputs(x[2 * c : 2 * c + 2], w_qkv, b_qkv, w_proj, b_proj)
        for c in range(8)
    ]
    res = run_bass_kernel_spmd(nc, in_maps, core_ids=list(range(8)))
    out = np.empty((B, N, Dd), dtype=np.float32)
    for c in range(8):
        out[2 * c : 2 * c + 2] = res.results[c]["out"].reshape(2, N, Dd)
    kernel.last_results = res
    return out
